# revision 53
# baseline (speedup 1.0000x reference)
"""Bass/Trainium2 kernel for GQA attention block (nn_FP8Attention).

Full-input contract: kernel(**inputs) takes the complete unsharded inputs and
returns the full [B, S, HIDDEN] output. Internally shards across 8 NeuronCores
as (batch, kv-head-group) pairs: each core handles 1 batch, 1 KV head and its
4 Q heads, computes attention for all 2048 tokens of its batch, then computes
the partial output projection through its heads' rows of wo per 512-token
window and ReduceScatters (sum) the partials within each batch's 4-core group,
leaving each core with the final output for 4x128 of its batch's tokens.

vs the original AllToAll design: x is shipped pre-transposed/pre-cast (no
on-device transposes or f32->bf16 casts), wo is sharded by head rows
(2MB/core instead of full 8MB), and the 8-way AllToAll + staging + full
o-proj tail is replaced by per-window partial o-proj + 4-way ReduceScatter
overlapped with later windows' compute. On-core scheduling: attention scores
run two k-blocks ahead of the PV accumulates (PE never waits on the ACT
exp), causal masking is a post-exp 0/1 multiply on the otherwise-idle gpsimd
queue, the softmax denominator is accumulated elementwise on DVE and
partition-summed by a single ones-matmul per head (instead of re-streaming
every exp block through the PE), each head's normalization chain is emitted
inside the next head's score stream, V is transposed by XBAR DMA from the SP
queue, and PSUM eviction copies ride the ACT engine. Modeled single-core
time: 261us vs 438us for the original (PE-bound, ~78% busy).
"""

import math
import sys
from collections import deque

for _p in ("/opt/trn_rl_repo",):
    if _p not in sys.path:
        sys.path.insert(0, _p)

import numpy as np
import ml_dtypes

import concourse.bass as bass
import concourse.mybir as mybir
import concourse.tile as tile
from concourse import bacc
from concourse.bass_utils import run_bass_kernel_spmd

BF16 = ml_dtypes.bfloat16

B, S, H = 2, 2048, 2048
NH, NKV, HD = 16, 4, 128
P = 128
THETA = 10000.0
NCORES = 8
N_RS = int(__import__("os").environ.get("KERNEL_NRS", "4"))
SW = S // 4          # tokens owned per core after ReduceScatter (512)
ISQ = 1.0 / math.sqrt(HD)
HIDC = H // P        # 16 hidden chunks
QHEADS = 4           # q heads per core


def _emit(tc, aps):
    nc = tc.nc
    f32 = mybir.dt.float32
    bf16 = mybir.dt.bfloat16
    Exp = mybir.ActivationFunctionType.Exp

    xT = aps["xT"]
    wqkvT = aps["wqkvT"]
    woTh = aps["woTh"]
    cos_t = aps["cos_t"]
    sin_t = aps["sin_t"]
    rotT = aps["rotT"]
    tri01 = aps["tri01"]
    ones_t = aps["ones_t"]
    padb = aps["padb"]
    y = aps["y"]

    xT_v = xT.rearrange("(hc p) t -> hc p t", p=P)
    wqkvT_v = wqkvT.rearrange("(hc p) o -> hc p o", p=P)
    woTh_v = woTh.rearrange("(h p) o -> h p o", p=P)

    with tc.tile_pool(name="consts", bufs=1) as cp:
        rot_sb = cp.tile([P, P], bf16)
        nc.sync.dma_start(rot_sb, rotT)
        tri01_sb = cp.tile([P, P], bf16)
        nc.sync.dma_start(tri01_sb, tri01)
        ones_sb = cp.tile([P, P], bf16)
        nc.sync.dma_start(ones_sb, ones_t)
        padb_sb = cp.tile([P, HIDC], f32)
        nc.sync.dma_start(padb_sb, padb)

        # weights + full xT resident in SBUF; window-0 token slices first so
        # the first projection can start before the bulk of x has landed
        wqkv_sb = cp.tile([P, HIDC, 768], bf16)
        xT_sb = cp.tile([P, HIDC, S], bf16)
        cos_sb = cp.tile([P, S], bf16)
        sin_sb = cp.tile([P, S], bf16)
        # interleave per-hc weight + window-0 x loads so the first projection
        # accumulation can chase the DMA stream instead of waiting for all;
        # window-0 cos/sin slices ride along early for the first rope
        for hc in range(HIDC):
            nc.sync.dma_start(wqkv_sb[:, hc, :], wqkvT_v[hc])
            nc.sync.dma_start(xT_sb[:, hc, 0:512], xT_v[hc][:, 0:512])
            if hc == 7:
                nc.sync.dma_start(cos_sb[:, 0:512], cos_t[:, 0:512])
                nc.sync.dma_start(sin_sb[:, 0:512], sin_t[:, 0:512])
        nc.sync.dma_start(cos_sb[:, 512:2048], cos_t[:, 512:2048])
        nc.sync.dma_start(sin_sb[:, 512:2048], sin_t[:, 512:2048])
        for hc in range(HIDC):
            nc.sync.dma_start(xT_sb[:, hc, 512:2048], xT_v[hc][:, 512:2048])
        woTh_sb = cp.tile([P, QHEADS, H], bf16)
        for h in range(QHEADS):
            nc.sync.dma_start(woTh_sb[:, h, :], woTh_v[h])

        # per-window activation tiles; nrm is per-(window, head) so the
        # o-projection's first matmuls don't wait on the last head's norm
        qk = [cp.tile([P, 5, 512], bf16, name=f"qk{w}") for w in range(4)]
        vn = [cp.tile([P, 512], bf16, name=f"vn{w}") for w in range(4)]
        nrm = [[cp.tile([P, 512], bf16, name=f"nrm{w}_{h}") for h in range(QHEADS)]
               for w in range(4)]

        with (
            tc.tile_pool(name="psA", bufs=1, space="PSUM") as psA,
            tc.tile_pool(name="rsd", bufs=1, space="DRAM") as rsd,
            tc.tile_pool(name="ph1", bufs=3) as ph1,
            tc.tile_pool(name="att", bufs=4) as att,
        ):
            if N_RS == 4:
                rs_in = [rsd.tile([4 * P, H], bf16, name=f"rs{w}")
                         for w in range(4)]
                rs_out = [rsd.tile([P, H], bf16, name=f"rso{w}")
                          for w in range(4)]
            else:
                # merged layout: receiver block g' = rows [512g', 512g'+512),
                # window w at rows [512g' + 128w, ...+128) -- one collective
                rs_in_all = rsd.tile([4 * SW, H], bf16, name="rs_all")
                rs_out_all = rsd.tile([SW, H], bf16, name="rso_all")

            for w in range(4):
                tw = slice(w * 512, (w + 1) * 512)
                # ---- QKV projections for this token window
                for oc in range(6):
                    ps_p = psA.tile([P, 512], f32, tag="proj", bufs=2)
                    for hc in range(HIDC):
                        nc.tensor.matmul(
                            ps_p,
                            lhsT=wqkv_sb[:, hc, oc * P:(oc + 1) * P],
                            rhs=xT_sb[:, hc, tw],
                            start=(hc == 0),
                            stop=(hc == HIDC - 1),
                        )
                    if oc < 5:
                        # RoPE: out = q*cos + rot(q)*sin, rot via PE matmul
                        raw = ph1.tile([P, 512], bf16, tag="raw")
                        nc.scalar.copy(raw, ps_p)
                        ps_r = psA.tile([P, 512], f32, tag="rot", bufs=1)
                        nc.tensor.matmul(
                            ps_r, lhsT=rot_sb, rhs=raw, start=True, stop=True
                        )
                        t1 = ph1.tile([P, 512], bf16, tag="t1")
                        nc.vector.tensor_mul(t1, ps_p, cos_sb[:, tw])
                        t2 = ph1.tile([P, 512], bf16, tag="t2")
                        nc.vector.tensor_mul(t2, ps_r, sin_sb[:, tw])
                        nc.vector.tensor_add(qk[w][:, oc, :], t1, t2)
                    else:
                        # V: evict V^T then XBAR DMA-transpose each 128x128
                        # block to natural [tok, hd] layout. Issued from the
                        # idle SP queue; frees a PSUM bank for a third score
                        # buffer and takes the transposes off the PE.
                        vTs = ph1.tile([P, 512], bf16, tag="vT")
                        nc.scalar.copy(vTs, ps_p)
                        for t in range(4):
                            nc.sync.dma_start_transpose(
                                vn[w][:, t * P:(t + 1) * P],
                                vTs[:, t * P:(t + 1) * P],
                            )
                # ---- attention column qc == w for all 4 heads
                qc = w
                n_kc = 4 * qc + 4
                def make_head(h):
                    ps_o = psA.tile([P, 512], f32, tag="o", bufs=2, name="ps_o")
                    # ps_d shares the rope bank: rope uses it only during the
                    # projection phase, ps_d only during attention
                    ps_d = psA.tile([P, 512], f32, tag="rot", bufs=1, name="ps_d")
                    # softmax denominator: accumulate exp blocks elementwise
                    # on DVE (d_acc[p, q] collects k = 128*kc + p), then one
                    # ones-matmul in finalize() does the partition sum --
                    # instead of re-streaming every exp block through the PE.
                    d_acc = att.tile([P, 512], f32, tag="d_acc", bufs=2)

                    def emit_score(kc, first):
                        """scores + exp for one 128-token k block; returns pT.

                        Causal masking: the 128x128 diagonal block is zeroed
                        AFTER exp by a 0/1 triangular multiply on gpsimd (an
                        otherwise idle queue), keeping the PE->ACT exp chain
                        free of DVE round-trips.
                        """
                        b0 = max(0, (kc - 4 * qc) * P)
                        N = 512 - b0
                        kw, kt = divmod(kc, 4)
                        ps_s = psA.tile([P, 512], f32, tag="s", bufs=3)
                        nc.tensor.matmul(
                            ps_s[:, :N],
                            lhsT=qk[kw][:, 4, kt * P:(kt + 1) * P],
                            rhs=qk[qc][:, h, b0:512],
                            start=True,
                            stop=True,
                        )
                        pT = att.tile([P, 512], bf16, tag="pT", bufs=6)
                        nc.scalar.activation(
                            pT[:, :N],
                            ps_s[:, :N],
                            Exp,
                            scale=ISQ,
                            bias=padb_sb[:, kc:kc + 1],
                        )
                        if kc >= 4 * qc:
                            nc.gpsimd.tensor_mul(
                                pT[:, 0:P], pT[:, 0:P], tri01_sb
                            )
                        if first:
                            nc.vector.tensor_copy(d_acc, pT)
                        else:
                            nc.vector.tensor_add(
                                d_acc[:, b0:512], d_acc[:, b0:512], pT[:, :N]
                            )
                        return pT, b0, N, kw, kt

                    def emit_accum(pT, b0, N, kw, kt, first, last):
                        nc.tensor.matmul(
                            ps_o[:, b0:512],
                            lhsT=vn[kw][:, kt * P:(kt + 1) * P],
                            rhs=pT[:, :N],
                            start=first,
                            stop=last,
                            skip_group_check=True,
                        )

                    def run(finalize_prev):
                        # software pipeline: scores run two k-blocks ahead of
                        # the o accumulates so PE never waits on the ACT exp;
                        # the previous head's normalization chain is emitted a
                        # couple of iterations in so its DVE work overlaps
                        # this head's matmuls instead of stalling the PE queue.
                        order = list(range(n_kc))
                        LOOKAHEAD = 2
                        pend = deque()
                        done_fin = finalize_prev is None
                        for pos, kc in enumerate(order):
                            pend.append((emit_score(kc, pos == 0), pos))
                            if len(pend) > LOOKAHEAD:
                                args, p0 = pend.popleft()
                                emit_accum(*args, p0 == 0, p0 == n_kc - 1)
                            if not done_fin and pos >= 1:
                                finalize_prev()
                                done_fin = True
                        if not done_fin:
                            finalize_prev()
                        while pend:
                            args, p0 = pend.popleft()
                            emit_accum(*args, p0 == 0, p0 == n_kc - 1)

                    def finalize():
                        # partition-sum of d_acc via one ones-matmul (bf16
                        # copy first: fp32 rhs would run the PE in fp32 mode)
                        d16 = att.tile([P, 512], bf16, tag="d16", bufs=2)
                        nc.vector.tensor_copy(d16, d_acc)
                        nc.tensor.matmul(
                            ps_d, lhsT=ones_sb, rhs=d16, start=True, stop=True
                        )
                        rec = att.tile([P, 512], f32, tag="rec", bufs=2)
                        nc.vector.reciprocal(rec, ps_d)
                        nc.vector.tensor_mul(nrm[w][h], ps_o, rec)

                    return run, finalize

                fin_prev = None
                for h in range(QHEADS):
                    run_head, fin = make_head(h)
                    run_head(fin_prev)
                    fin_prev = fin
                # ---- partial o-proj through this core's 4 head rows of wo.
                # ps_y pairs share the two "s" PSUM banks (free after the
                # attention scores above), and each pair interleaves its
                # h=0..2 accumulations before either h=3 so the PE doesn't
                # sit on the DVE latency of the last head's norm.
                yw_tiles = {}
                pairs = [((sub, fs), (sub, fs + 1))
                         for sub in range(4) for fs in (0, 2)]
                for pi, (gA, gB) in enumerate(pairs):
                    ps = {}
                    for g in (gA, gB):
                        ps[g] = psA.tile(
                            [P, 512], f32, tag="s", bufs=3, name="ps_y"
                        )
                    for h in range(QHEADS - 1):
                        for g in (gA, gB):
                            sub, fs = g
                            nc.tensor.matmul(
                                ps[g],
                                lhsT=nrm[w][h][:, sub * P:(sub + 1) * P],
                                rhs=woTh_sb[:, h, fs * 512:(fs + 1) * 512],
                                start=(h == 0),
                                stop=False,
                                skip_group_check=True,
                            )
                        if pi == 0 and h == 1 and fin_prev is not None:
                            # head 3's normalization chain lands here, hidden
                            # behind the first o-proj accumulations
                            fin_prev()
                            fin_prev = None
                    for g in (gA, gB):
                        sub, fs = g
                        h = QHEADS - 1
                        nc.tensor.matmul(
                            ps[g],
                            lhsT=nrm[w][h][:, sub * P:(sub + 1) * P],
                            rhs=woTh_sb[:, h, fs * 512:(fs + 1) * 512],
                            start=False,
                            stop=True,
                            skip_group_check=True,
                        )
                        if sub not in yw_tiles:
                            yw_tiles[sub] = ph1.tile(
                                [P, H], bf16, tag="yw", bufs=2, name="yw"
                            )
                        yw = yw_tiles[sub]
                        nc.scalar.copy(yw[:, fs * 512:(fs + 1) * 512], ps[g])
                        if fs == 3:
                            if N_RS == 4:
                                nc.sync.dma_start(
                                    rs_in[w][sub * P:(sub + 1) * P, :], yw
                                )
                            else:
                                r0 = 512 * sub + P * w
                                nc.sync.dma_start(rs_in_all[r0:r0 + P, :], yw)
                            del yw_tiles[sub]
                # ---- ReduceScatter within the 4-core batch group: receiver g
                # gets sum of partials for tokens [512w + 128g, 512w + 128g+128)
                if N_RS == 4:
                    if aps.get("_single_core"):
                        # timeline-sim stand-in for the collective
                        nc.sync.dma_start(rs_out[w], rs_in[w][0:P, :])
                    else:
                        nc.gpsimd.collective_compute(
                            "ReduceScatter",
                            mybir.AluOpType.add,
                            replica_groups=[[0, 1, 2, 3], [4, 5, 6, 7]],
                            ins=[rs_in[w].opt()],
                            outs=[rs_out[w].opt()],
                        )
                    nc.sync.dma_start(y[w * P:(w + 1) * P, :], rs_out[w])

            if N_RS == 1:
                if aps.get("_single_core"):
                    nc.sync.dma_start(rs_out_all, rs_in_all[0:SW, :])
                else:
                    nc.gpsimd.collective_compute(
                        "ReduceScatter",
                        mybir.AluOpType.add,
                        replica_groups=[[0, 1, 2, 3], [4, 5, 6, 7]],
                        ins=[rs_in_all.opt()],
                        outs=[rs_out_all.opt()],
                    )
                nc.sync.dma_start(y, rs_out_all)


def build_nc(debug=False, single_core=False):
    nc = bacc.Bacc(
        "TRN2",
        target_bir_lowering=False,
        debug=debug,
        enable_asserts=True,
        num_devices=1 if single_core else NCORES,
    )
    f32 = mybir.dt.float32
    bf16 = mybir.dt.bfloat16
    aps = {
        "xT": nc.dram_tensor("xT", [H, S], bf16, kind="ExternalInput").ap(),
        "wqkvT": nc.dram_tensor("wqkvT", [H, 768], bf16, kind="ExternalInput").ap(),
        "woTh": nc.dram_tensor("woTh", [512, H], bf16, kind="ExternalInput").ap(),
        "cos_t": nc.dram_tensor("cos_t", [P, S], bf16, kind="ExternalInput").ap(),
        "sin_t": nc.dram_tensor("sin_t", [P, S], bf16, kind="ExternalInput").ap(),
        "rotT": nc.dram_tensor("rotT", [P, P], bf16, kind="ExternalInput").ap(),
        "tri01": nc.dram_tensor("tri01", [P, P], bf16, kind="ExternalInput").ap(),
        "ones_t": nc.dram_tensor("ones_t", [P, P], bf16, kind="ExternalInput").ap(),
        "padb": nc.dram_tensor("padb", [P, HIDC], f32, kind="ExternalInput").ap(),
        "y": nc.dram_tensor("y", [SW, H], bf16, kind="ExternalOutput").ap(),
    }
    if single_core:
        aps["_single_core"] = True
    with tile.TileContext(nc) as tc:
        _emit(tc, aps)
    nc.compile()
    return nc


def _to_bf16(a):
    """Fast f32 -> bf16 cast (round-to-nearest-even) via bit manipulation."""
    u = np.ascontiguousarray(a, dtype=np.float32).view(np.uint32)
    r = ((u >> 16) & 1) + np.uint32(0x7FFF)
    return ((u + r) >> 16).astype(np.uint16).view(BF16)


_CONSTS = {}


def _const_tables():
    if _CONSTS:
        return _CONSTS
    pos = np.arange(S, dtype=np.float32)
    inv = 1.0 / THETA ** (np.arange(0, HD, 2, dtype=np.float32) / HD)  # [64]
    ang = inv[:, None] * pos[None, :]                 # [64, S]
    _CONSTS["cos_t"] = np.concatenate(
        [np.cos(ang), np.cos(ang)], axis=0).astype(BF16)
    _CONSTS["sin_t"] = np.concatenate(
        [np.sin(ang), np.sin(ang)], axis=0).astype(BF16)
    A = np.zeros((P, P), dtype=np.float32)
    i = np.arange(64)
    A[i, i + 64] = -1.0
    A[i + 64, i] = 1.0
    _CONSTS["rotT"] = np.ascontiguousarray(A.T).astype(BF16)
    # 0/1 keep-mask for the causal diagonal block: keep q >= k
    _CONSTS["tri01"] = np.where(
        np.arange(P)[None, :] >= np.arange(P)[:, None], 1.0, 0.0
    ).astype(BF16)
    _CONSTS["ones_t"] = np.ones((P, P), dtype=BF16)
    return _CONSTS


def host_inputs(hidden_states, attention_mask, wq, wk, wv, wo):
    """Build the per-core input maps (host-side sharding + constant tables)."""
    hs = np.asarray(hidden_states, dtype=np.float32)
    am = np.asarray(attention_mask)
    wq = np.asarray(wq, dtype=np.float32)
    wk = np.asarray(wk, dtype=np.float32)
    wv = np.asarray(wv, dtype=np.float32)
    wo = np.asarray(wo, dtype=np.float32)
    C = _const_tables()

    # per-batch: pre-transposed bf16 activations + pad bias (shared by 4 cores)
    xT_b, padb_b = [], []
    for b in range(B):
        xT_b.append(np.ascontiguousarray(_to_bf16(hs[b]).T))
        padb = np.where(
            am[b].astype(bool), 0.0, -1e30
        ).astype(np.float32).reshape(HIDC, P).T          # [P, HIDC]
        padb_b.append(np.ascontiguousarray(padb))

    # per-group: qkv + wo-rows weight slices (shared by both batches)
    wqkvT_g, woTh_g = [], []
    for g in range(NKV):
        wqT = wq[4 * g * HD:(4 * g + 4) * HD, :].T       # [H, 512]
        wkT = wk[g * HD:(g + 1) * HD, :].T               # [H, 128]
        wvT = wv[g * HD:(g + 1) * HD, :].T               # [H, 128]
        wqkvT_g.append(np.ascontiguousarray(
            np.concatenate([wqT, wkT, wvT], axis=1)).astype(BF16))
        woTh_g.append(
            np.ascontiguousarray(wo[:, 4 * g * HD:(4 * g + 4) * HD].T).astype(BF16))

    in_maps = []
    for core in range(NCORES):
        b, g = divmod(core, 4)
        in_maps.append(
            {
                "xT": xT_b[b],
                "wqkvT": wqkvT_g[g],
                "woTh": woTh_g[g],
                "cos_t": C["cos_t"],
                "sin_t": C["sin_t"],
                "rotT": C["rotT"],
                "tri01": C["tri01"],
                "ones_t": C["ones_t"],
                "padb": padb_b[b],
            }
        )
    return in_maps


def assemble(results):
    """Gather per-core outputs into the full [B, S, H] array.

    Core (b, g) owns tokens {512*w + 128*g + i} for w in 0..3: its y row
    block w holds the ReduceScattered (summed) output for those tokens.
    """
    out = np.empty((B, S, H), dtype=np.float32)
    for core in range(NCORES):
        b, g = divmod(core, 4)
        yc = np.asarray(results[core]["y"], dtype=np.float32)
        for w in range(4):
            r0 = 512 * w + 128 * g
            out[b, r0:r0 + P, :] = yc[w * P:(w + 1) * P, :]
    return out


_NC_CACHE = {}


def kernel(hidden_states, attention_mask, wq, wk, wv, wo, **run_kwargs):
    in_maps = host_inputs(hidden_states, attention_mask, wq, wk, wv, wo)
    if "nc" not in _NC_CACHE:
        _NC_CACHE["nc"] = build_nc(debug=False)
    nc = _NC_CACHE["nc"]
    res = run_bass_kernel_spmd(nc, in_maps, core_ids=list(range(NCORES)), **run_kwargs)
    out = assemble(res.results)
    kernel.last_results = res
    return out


# revision 58
# speedup vs baseline: 1.0057x; 1.0057x over previous
"""Bass/Trainium2 kernel for GQA attention block (nn_FP8Attention).

Full-input contract: kernel(**inputs) takes the complete unsharded inputs and
returns the full [B, S, HIDDEN] output. Internally shards across 8 NeuronCores
as (batch, kv-head-group) pairs: each core handles 1 batch, 1 KV head and its
4 Q heads, computes attention for all 2048 tokens of its batch, then computes
the partial output projection through its heads' rows of wo per 512-token
window and ReduceScatters (sum) the partials within each batch's 4-core group,
leaving each core with the final output for 4x128 of its batch's tokens.

vs the original AllToAll design: x is shipped pre-transposed/pre-cast (no
on-device transposes or f32->bf16 casts), wo is sharded by head rows
(2MB/core instead of full 8MB), and the 8-way AllToAll + staging + full
o-proj tail is replaced by per-window partial o-proj + 4-way ReduceScatter
overlapped with later windows' compute. On-core scheduling: attention scores
run two k-blocks ahead of the PV accumulates (PE never waits on the ACT
exp), causal masking is a post-exp 0/1 multiply on the otherwise-idle gpsimd
queue, the softmax denominator is accumulated elementwise on DVE and
partition-summed by a single ones-matmul per head (instead of re-streaming
every exp block through the PE), each head's normalization chain is emitted
inside the next head's score stream, V is transposed by XBAR DMA from the SP
queue, and PSUM eviction copies ride the ACT engine. Modeled single-core
time: 261us vs 438us for the original (PE-bound, ~78% busy).
"""

import math
import sys
from collections import deque

for _p in ("/opt/trn_rl_repo",):
    if _p not in sys.path:
        sys.path.insert(0, _p)

import numpy as np
import ml_dtypes

import concourse.bass as bass
import concourse.mybir as mybir
import concourse.tile as tile
from concourse import bacc
from concourse.bass_utils import run_bass_kernel_spmd

BF16 = ml_dtypes.bfloat16

B, S, H = 2, 2048, 2048
NH, NKV, HD = 16, 4, 128
P = 128
THETA = 10000.0
NCORES = 8
N_RS = int(__import__("os").environ.get("KERNEL_NRS", "4"))
# timing diagnostic ONLY: replaces collectives with local DMA (wrong output
# for 3/4 of rows) to isolate collective cost from launch/compute cost
NO_CC = __import__("os").environ.get("KERNEL_NOCC", "") == "1"

SW = S // 4          # tokens owned per core after ReduceScatter (512)
ISQ = 1.0 / math.sqrt(HD)
HIDC = H // P        # 16 hidden chunks
QHEADS = 4           # q heads per core


def _emit(tc, aps):
    nc = tc.nc
    f32 = mybir.dt.float32
    bf16 = mybir.dt.bfloat16
    Exp = mybir.ActivationFunctionType.Exp

    xT = aps["xT"]
    wqkvT = aps["wqkvT"]
    woTh = aps["woTh"]
    cos_t = aps["cos_t"]
    sin_t = aps["sin_t"]
    rotT = aps["rotT"]
    tri01 = aps["tri01"]
    ones_t = aps["ones_t"]
    padb = aps["padb"]
    y = aps["y"]

    xT_v = xT.rearrange("(hc p) t -> hc p t", p=P)
    wqkvT_v = wqkvT.rearrange("(hc p) o -> hc p o", p=P)
    woTh_v = woTh.rearrange("(h p) o -> h p o", p=P)

    with tc.tile_pool(name="consts", bufs=1) as cp:
        rot_sb = cp.tile([P, P], bf16)
        nc.sync.dma_start(rot_sb, rotT)
        tri01_sb = cp.tile([P, P], bf16)
        nc.sync.dma_start(tri01_sb, tri01)
        ones_sb = cp.tile([P, P], bf16)
        nc.sync.dma_start(ones_sb, ones_t)
        padb_sb = cp.tile([P, HIDC], f32)
        nc.sync.dma_start(padb_sb, padb)

        # weights + full xT resident in SBUF; window-0 token slices first so
        # the first projection can start before the bulk of x has landed
        wqkv_sb = cp.tile([P, HIDC, 768], bf16)
        xT_sb = cp.tile([P, HIDC, S], bf16)
        cos_sb = cp.tile([P, S], bf16)
        sin_sb = cp.tile([P, S], bf16)
        # interleave per-hc weight + window-0 x loads so the first projection
        # accumulation can chase the DMA stream instead of waiting for all;
        # window-0 cos/sin slices ride along early for the first rope
        for hc in range(HIDC):
            nc.sync.dma_start(wqkv_sb[:, hc, :], wqkvT_v[hc])
            nc.sync.dma_start(xT_sb[:, hc, 0:512], xT_v[hc][:, 0:512])
            if hc == 7:
                nc.sync.dma_start(cos_sb[:, 0:512], cos_t[:, 0:512])
                nc.sync.dma_start(sin_sb[:, 0:512], sin_t[:, 0:512])
        nc.sync.dma_start(cos_sb[:, 512:2048], cos_t[:, 512:2048])
        nc.sync.dma_start(sin_sb[:, 512:2048], sin_t[:, 512:2048])
        for hc in range(HIDC):
            nc.sync.dma_start(xT_sb[:, hc, 512:2048], xT_v[hc][:, 512:2048])
        woTh_sb = cp.tile([P, QHEADS, H], bf16)
        for h in range(QHEADS):
            nc.sync.dma_start(woTh_sb[:, h, :], woTh_v[h])

        # per-window activation tiles; nrm is per-(window, head) so the
        # o-projection's first matmuls don't wait on the last head's norm
        qk = [cp.tile([P, 5, 512], bf16, name=f"qk{w}") for w in range(4)]
        vn = [cp.tile([P, 512], bf16, name=f"vn{w}") for w in range(4)]
        nrm = [[cp.tile([P, 512], bf16, name=f"nrm{w}_{h}") for h in range(QHEADS)]
               for w in range(4)]

        with (
            tc.tile_pool(name="psA", bufs=1, space="PSUM") as psA,
            tc.tile_pool(name="rsd", bufs=1, space="DRAM") as rsd,
            tc.tile_pool(name="ph1", bufs=3) as ph1,
            tc.tile_pool(name="att", bufs=4) as att,
        ):
            if N_RS == 4:
                rs_in = [rsd.tile([4 * P, H], bf16, name=f"rs{w}")
                         for w in range(4)]
                rs_out = [rsd.tile([P, H], bf16, name=f"rso{w}")
                          for w in range(4)]
            else:
                # merged layout: receiver block g' = rows [512g', 512g'+512),
                # window w at rows [512g' + 128w, ...+128) -- one collective
                rs_in_all = rsd.tile([4 * SW, H], bf16, name="rs_all")
                rs_out_all = rsd.tile([SW, H], bf16, name="rso_all")

            for w in range(4):
                tw = slice(w * 512, (w + 1) * 512)
                # ---- QKV projections for this token window
                for oc in range(6):
                    ps_p = psA.tile([P, 512], f32, tag="proj", bufs=2)
                    for hc in range(HIDC):
                        nc.tensor.matmul(
                            ps_p,
                            lhsT=wqkv_sb[:, hc, oc * P:(oc + 1) * P],
                            rhs=xT_sb[:, hc, tw],
                            start=(hc == 0),
                            stop=(hc == HIDC - 1),
                        )
                    if oc < 5:
                        # RoPE: out = q*cos + rot(q)*sin, rot via PE matmul
                        raw = ph1.tile([P, 512], bf16, tag="raw")
                        nc.scalar.copy(raw, ps_p)
                        ps_r = psA.tile([P, 512], f32, tag="rot", bufs=1)
                        nc.tensor.matmul(
                            ps_r, lhsT=rot_sb, rhs=raw, start=True, stop=True
                        )
                        t1 = ph1.tile([P, 512], bf16, tag="t1")
                        nc.vector.tensor_mul(t1, ps_p, cos_sb[:, tw])
                        t2 = ph1.tile([P, 512], bf16, tag="t2")
                        nc.vector.tensor_mul(t2, ps_r, sin_sb[:, tw])
                        nc.vector.tensor_add(qk[w][:, oc, :], t1, t2)
                    else:
                        # V: evict V^T then XBAR DMA-transpose each 128x128
                        # block to natural [tok, hd] layout. Issued from the
                        # idle SP queue; frees a PSUM bank for a third score
                        # buffer and takes the transposes off the PE.
                        vTs = ph1.tile([P, 512], bf16, tag="vT")
                        nc.scalar.copy(vTs, ps_p)
                        for t in range(4):
                            nc.sync.dma_start_transpose(
                                vn[w][:, t * P:(t + 1) * P],
                                vTs[:, t * P:(t + 1) * P],
                            )
                # ---- attention column qc == w for all 4 heads
                qc = w
                n_kc = 4 * qc + 4
                def make_head(h):
                    ps_o = psA.tile([P, 512], f32, tag="o", bufs=2, name="ps_o")
                    # ps_d shares the rope bank: rope uses it only during the
                    # projection phase, ps_d only during attention
                    ps_d = psA.tile([P, 512], f32, tag="rot", bufs=1, name="ps_d")
                    # softmax denominator: accumulate exp blocks elementwise
                    # on DVE (d_acc[p, q] collects k = 128*kc + p), then one
                    # ones-matmul in finalize() does the partition sum --
                    # instead of re-streaming every exp block through the PE.
                    d_acc = att.tile([P, 512], f32, tag="d_acc", bufs=2)

                    def emit_score(kc, first):
                        """scores + exp for one 128-token k block; returns pT.

                        Causal masking: the 128x128 diagonal block is zeroed
                        AFTER exp by a 0/1 triangular multiply on gpsimd (an
                        otherwise idle queue), keeping the PE->ACT exp chain
                        free of DVE round-trips.
                        """
                        b0 = max(0, (kc - 4 * qc) * P)
                        N = 512 - b0
                        kw, kt = divmod(kc, 4)
                        ps_s = psA.tile([P, 512], f32, tag="s", bufs=3)
                        nc.tensor.matmul(
                            ps_s[:, :N],
                            lhsT=qk[kw][:, 4, kt * P:(kt + 1) * P],
                            rhs=qk[qc][:, h, b0:512],
                            start=True,
                            stop=True,
                        )
                        pT = att.tile([P, 512], bf16, tag="pT", bufs=6)
                        nc.scalar.activation(
                            pT[:, :N],
                            ps_s[:, :N],
                            Exp,
                            scale=ISQ,
                            bias=padb_sb[:, kc:kc + 1],
                        )
                        if kc >= 4 * qc:
                            nc.gpsimd.tensor_mul(
                                pT[:, 0:P], pT[:, 0:P], tri01_sb
                            )
                        if first:
                            nc.vector.tensor_copy(d_acc, pT)
                        else:
                            nc.vector.tensor_add(
                                d_acc[:, b0:512], d_acc[:, b0:512], pT[:, :N]
                            )
                        return pT, b0, N, kw, kt

                    def emit_accum(pT, b0, N, kw, kt, first, last):
                        nc.tensor.matmul(
                            ps_o[:, b0:512],
                            lhsT=vn[kw][:, kt * P:(kt + 1) * P],
                            rhs=pT[:, :N],
                            start=first,
                            stop=last,
                            skip_group_check=True,
                        )

                    def run(finalize_prev):
                        # software pipeline: scores run two k-blocks ahead of
                        # the o accumulates so PE never waits on the ACT exp;
                        # the previous head's normalization chain is emitted a
                        # couple of iterations in so its DVE work overlaps
                        # this head's matmuls instead of stalling the PE queue.
                        order = list(range(n_kc))
                        LOOKAHEAD = 2
                        pend = deque()
                        done_fin = finalize_prev is None
                        for pos, kc in enumerate(order):
                            pend.append((emit_score(kc, pos == 0), pos))
                            if len(pend) > LOOKAHEAD:
                                args, p0 = pend.popleft()
                                emit_accum(*args, p0 == 0, p0 == n_kc - 1)
                            if not done_fin and pos >= 1:
                                finalize_prev()
                                done_fin = True
                        if not done_fin:
                            finalize_prev()
                        while pend:
                            args, p0 = pend.popleft()
                            emit_accum(*args, p0 == 0, p0 == n_kc - 1)

                    def finalize():
                        # partition-sum of d_acc via one ones-matmul (bf16
                        # copy first: fp32 rhs would run the PE in fp32 mode)
                        d16 = att.tile([P, 512], bf16, tag="d16", bufs=2)
                        nc.vector.tensor_copy(d16, d_acc)
                        nc.tensor.matmul(
                            ps_d, lhsT=ones_sb, rhs=d16, start=True, stop=True
                        )
                        rec = att.tile([P, 512], f32, tag="rec", bufs=2)
                        nc.vector.reciprocal(rec, ps_d)
                        nc.vector.tensor_mul(nrm[w][h], ps_o, rec)

                    return run, finalize

                fin_prev = None
                for h in range(QHEADS):
                    run_head, fin = make_head(h)
                    run_head(fin_prev)
                    fin_prev = fin
                # ---- partial o-proj through this core's 4 head rows of wo.
                # ps_y pairs share the two "s" PSUM banks (free after the
                # attention scores above), and each pair interleaves its
                # h=0..2 accumulations before either h=3 so the PE doesn't
                # sit on the DVE latency of the last head's norm.
                yw_tiles = {}
                pairs = [((sub, fs), (sub, fs + 1))
                         for sub in range(4) for fs in (0, 2)]
                for pi, (gA, gB) in enumerate(pairs):
                    ps = {}
                    for g in (gA, gB):
                        ps[g] = psA.tile(
                            [P, 512], f32, tag="s", bufs=3, name="ps_y"
                        )
                    for h in range(QHEADS - 1):
                        for g in (gA, gB):
                            sub, fs = g
                            nc.tensor.matmul(
                                ps[g],
                                lhsT=nrm[w][h][:, sub * P:(sub + 1) * P],
                                rhs=woTh_sb[:, h, fs * 512:(fs + 1) * 512],
                                start=(h == 0),
                                stop=False,
                                skip_group_check=True,
                            )
                        if pi == 0 and h == 1 and fin_prev is not None:
                            # head 3's normalization chain lands here, hidden
                            # behind the first o-proj accumulations
                            fin_prev()
                            fin_prev = None
                    for g in (gA, gB):
                        sub, fs = g
                        h = QHEADS - 1
                        nc.tensor.matmul(
                            ps[g],
                            lhsT=nrm[w][h][:, sub * P:(sub + 1) * P],
                            rhs=woTh_sb[:, h, fs * 512:(fs + 1) * 512],
                            start=False,
                            stop=True,
                            skip_group_check=True,
                        )
                        if sub not in yw_tiles:
                            yw_tiles[sub] = ph1.tile(
                                [P, H], bf16, tag="yw", bufs=2, name="yw"
                            )
                        yw = yw_tiles[sub]
                        nc.scalar.copy(yw[:, fs * 512:(fs + 1) * 512], ps[g])
                        if fs == 3:
                            if N_RS == 4:
                                nc.sync.dma_start(
                                    rs_in[w][sub * P:(sub + 1) * P, :], yw
                                )
                            else:
                                r0 = 512 * sub + P * w
                                nc.sync.dma_start(rs_in_all[r0:r0 + P, :], yw)
                            del yw_tiles[sub]
                # ---- ReduceScatter within the 4-core batch group: receiver g
                # gets sum of partials for tokens [512w + 128g, 512w + 128g+128)
                if N_RS == 4:
                    if aps.get("_single_core") or NO_CC:
                        # timeline-sim stand-in for the collective
                        nc.sync.dma_start(rs_out[w], rs_in[w][0:P, :])
                    else:
                        nc.gpsimd.collective_compute(
                            "ReduceScatter",
                            mybir.AluOpType.add,
                            replica_groups=[[0, 1, 2, 3], [4, 5, 6, 7]],
                            ins=[rs_in[w].opt()],
                            outs=[rs_out[w].opt()],
                        )
                    nc.sync.dma_start(y[w * P:(w + 1) * P, :], rs_out[w])

            if N_RS == 1:
                if aps.get("_single_core") or NO_CC:
                    nc.sync.dma_start(rs_out_all, rs_in_all[0:SW, :])
                else:
                    nc.gpsimd.collective_compute(
                        "ReduceScatter",
                        mybir.AluOpType.add,
                        replica_groups=[[0, 1, 2, 3], [4, 5, 6, 7]],
                        ins=[rs_in_all.opt()],
                        outs=[rs_out_all.opt()],
                    )
                nc.sync.dma_start(y, rs_out_all)


def build_nc(debug=False, single_core=False):
    nc = bacc.Bacc(
        "TRN2",
        target_bir_lowering=False,
        debug=debug,
        enable_asserts=True,
        num_devices=1 if single_core else NCORES,
    )
    f32 = mybir.dt.float32
    bf16 = mybir.dt.bfloat16
    aps = {
        "xT": nc.dram_tensor("xT", [H, S], bf16, kind="ExternalInput").ap(),
        "wqkvT": nc.dram_tensor("wqkvT", [H, 768], bf16, kind="ExternalInput").ap(),
        "woTh": nc.dram_tensor("woTh", [512, H], bf16, kind="ExternalInput").ap(),
        "cos_t": nc.dram_tensor("cos_t", [P, S], bf16, kind="ExternalInput").ap(),
        "sin_t": nc.dram_tensor("sin_t", [P, S], bf16, kind="ExternalInput").ap(),
        "rotT": nc.dram_tensor("rotT", [P, P], bf16, kind="ExternalInput").ap(),
        "tri01": nc.dram_tensor("tri01", [P, P], bf16, kind="ExternalInput").ap(),
        "ones_t": nc.dram_tensor("ones_t", [P, P], bf16, kind="ExternalInput").ap(),
        "padb": nc.dram_tensor("padb", [P, HIDC], f32, kind="ExternalInput").ap(),
        "y": nc.dram_tensor("y", [SW, H], bf16, kind="ExternalOutput").ap(),
    }
    if single_core:
        aps["_single_core"] = True
    with tile.TileContext(nc) as tc:
        _emit(tc, aps)
    nc.compile()
    return nc


def _to_bf16(a):
    """Fast f32 -> bf16 cast (round-to-nearest-even) via bit manipulation."""
    u = np.ascontiguousarray(a, dtype=np.float32).view(np.uint32)
    r = ((u >> 16) & 1) + np.uint32(0x7FFF)
    return ((u + r) >> 16).astype(np.uint16).view(BF16)


_CONSTS = {}


def _const_tables():
    if _CONSTS:
        return _CONSTS
    pos = np.arange(S, dtype=np.float32)
    inv = 1.0 / THETA ** (np.arange(0, HD, 2, dtype=np.float32) / HD)  # [64]
    ang = inv[:, None] * pos[None, :]                 # [64, S]
    _CONSTS["cos_t"] = np.concatenate(
        [np.cos(ang), np.cos(ang)], axis=0).astype(BF16)
    _CONSTS["sin_t"] = np.concatenate(
        [np.sin(ang), np.sin(ang)], axis=0).astype(BF16)
    A = np.zeros((P, P), dtype=np.float32)
    i = np.arange(64)
    A[i, i + 64] = -1.0
    A[i + 64, i] = 1.0
    _CONSTS["rotT"] = np.ascontiguousarray(A.T).astype(BF16)
    # 0/1 keep-mask for the causal diagonal block: keep q >= k
    _CONSTS["tri01"] = np.where(
        np.arange(P)[None, :] >= np.arange(P)[:, None], 1.0, 0.0
    ).astype(BF16)
    _CONSTS["ones_t"] = np.ones((P, P), dtype=BF16)
    return _CONSTS


def host_inputs(hidden_states, attention_mask, wq, wk, wv, wo):
    """Build the per-core input maps (host-side sharding + constant tables)."""
    hs = np.asarray(hidden_states, dtype=np.float32)
    am = np.asarray(attention_mask)
    wq = np.asarray(wq, dtype=np.float32)
    wk = np.asarray(wk, dtype=np.float32)
    wv = np.asarray(wv, dtype=np.float32)
    wo = np.asarray(wo, dtype=np.float32)
    C = _const_tables()

    # per-batch: pre-transposed bf16 activations + pad bias (shared by 4 cores)
    xT_b, padb_b = [], []
    for b in range(B):
        xT_b.append(np.ascontiguousarray(_to_bf16(hs[b]).T))
        padb = np.where(
            am[b].astype(bool), 0.0, -1e30
        ).astype(np.float32).reshape(HIDC, P).T          # [P, HIDC]
        padb_b.append(np.ascontiguousarray(padb))

    # per-group: qkv + wo-rows weight slices (shared by both batches)
    wqkvT_g, woTh_g = [], []
    for g in range(NKV):
        wqT = wq[4 * g * HD:(4 * g + 4) * HD, :].T       # [H, 512]
        wkT = wk[g * HD:(g + 1) * HD, :].T               # [H, 128]
        wvT = wv[g * HD:(g + 1) * HD, :].T               # [H, 128]
        wqkvT_g.append(np.ascontiguousarray(
            np.concatenate([wqT, wkT, wvT], axis=1)).astype(BF16))
        woTh_g.append(
            np.ascontiguousarray(wo[:, 4 * g * HD:(4 * g + 4) * HD].T).astype(BF16))

    in_maps = []
    for core in range(NCORES):
        b, g = divmod(core, 4)
        in_maps.append(
            {
                "xT": xT_b[b],
                "wqkvT": wqkvT_g[g],
                "woTh": woTh_g[g],
                "cos_t": C["cos_t"],
                "sin_t": C["sin_t"],
                "rotT": C["rotT"],
                "tri01": C["tri01"],
                "ones_t": C["ones_t"],
                "padb": padb_b[b],
            }
        )
    return in_maps


def assemble(results):
    """Gather per-core outputs into the full [B, S, H] array.

    Core (b, g) owns tokens {512*w + 128*g + i} for w in 0..3: its y row
    block w holds the ReduceScattered (summed) output for those tokens.
    """
    out = np.empty((B, S, H), dtype=np.float32)
    for core in range(NCORES):
        b, g = divmod(core, 4)
        yc = np.asarray(results[core]["y"], dtype=np.float32)
        for w in range(4):
            r0 = 512 * w + 128 * g
            out[b, r0:r0 + P, :] = yc[w * P:(w + 1) * P, :]
    return out


_NC_CACHE = {}


def kernel(hidden_states, attention_mask, wq, wk, wv, wo, **run_kwargs):
    in_maps = host_inputs(hidden_states, attention_mask, wq, wk, wv, wo)
    if "nc" not in _NC_CACHE:
        _NC_CACHE["nc"] = build_nc(debug=False)
    nc = _NC_CACHE["nc"]
    res = run_bass_kernel_spmd(nc, in_maps, core_ids=list(range(NCORES)), **run_kwargs)
    out = assemble(res.results)
    kernel.last_results = res
    return out
